# revision 22
# baseline (speedup 1.0000x reference)
"""Trainium2 Bass kernel for nn_Net_12421045420310 (GNN edge-conditioned message passing).

Sharding (8 cores):
 - Nodes block-sharded: core c owns nodes [c*3125, (c+1)*3125).
 - Edges assigned to the core owning their dst node, sorted by dst within the
   shard -> scatter-mean is purely core-local; node state is re-replicated
   with one AllGather per conv step (bf16).
 - edge_index3/edge_attr3 position-sharded 5000/core; outputs stitched on host.

Device pipeline per conv step (per core):
 - Wedge ([128,64,64] per 128-edge tile) recomputed on TensorE from the
   SBUF-resident rh1^T (bf16) and a host-permuted f-major W_e2 (bf16, sharded
   across cores and AllGather'd on device at startup); never written to HBM.
 - out[src] rows gathered from a bf16 node-state replica via indirect DMA.
 - per-edge GEMV msg[e,f] = sum_d out_src[e,d]*Wedge[e,d,f] on VectorE:
   broadcast-AP tensor_tensor multiply + halving-add tree straight into bf16.
 - scatter-mean via selection-matrix matmuls on TensorE; the selection
   matrices are built on device (iota + per-partition is_equal compare) with
   1/cnt folded into their nonzeros, producing agg^T in [feat, node] layout.
 - GRU gates on PE/ScalarE/VectorE in transposed layout; h^T transposed back
   per 128-node chunk on TensorE, DMA'd to DRAM, AllGather.
"""

import hashlib
import math

import numpy as np
import ml_dtypes

import sys
import types

# This axon client build lacks antenv.axon_hooks; stub it so importing
# bass_utils under axon never trips on the optional profile hook.
if "antenv.axon_hooks" not in sys.modules:
    try:
        import antenv.axon_hooks  # noqa: F401
    except ImportError:
        _stub = types.ModuleType("antenv.axon_hooks")
        _stub.get_axon_ntff_profile_hook = lambda: None
        sys.modules["antenv.axon_hooks"] = _stub

import concourse.bass as bass
import concourse.bacc as bacc
import concourse.tile as tile
import concourse.mybir as mybir

AF = mybir.ActivationFunctionType
ALU = mybir.AluOpType
DT = mybir.dt

BF16 = DT.bfloat16
F32 = DT.float32
I32 = DT.int32

BF = ml_dtypes.bfloat16


class Cfg:
    def __init__(self, N=25000, E=50000, E3=40000, DIM=64, NCORES=8, K_SLOTS=3,
                 REPS=1, do_wedge=True, do_apply=True, do_gather=True,
                 no_coll=False):
        assert N % NCORES == 0 and E3 % NCORES == 0 and DIM == 64
        self.N, self.E, self.E3, self.DIM, self.NCORES = N, E, E3, DIM, NCORES
        self.NPC = N // NCORES                      # nodes per core
        self.NCH = math.ceil(self.NPC / 128)        # node chunks per core
        self.NPC_PAD = self.NCH * 128
        self.E3PC = E3 // NCORES
        self.NT3 = math.ceil(self.E3PC / 128)
        self.E3_PAD = self.NT3 * 128
        self.K_SLOTS = K_SLOTS
        self.REPS = REPS
        self.do_wedge = do_wedge
        self.do_apply = do_apply
        self.do_gather = do_gather
        self.no_coll = no_coll
        # filled by prep():
        self.NT_E = None
        self.EC_PAD = None
        self.has_be2 = False

    def slot_chunk(self, t, slot):
        """Node chunk targeted by scatter slot (t, slot); identical across cores."""
        c = t * self.NCH // self.NT_E + slot - (self.K_SLOTS // 2)
        return min(max(c, 0), self.NCH - 1)


def _remap_node(cfg, n):
    """Map global node ids -> rows in the interleaved padded replica layout."""
    n = np.asarray(n, np.int64)
    c, l = np.divmod(n, cfg.NPC)
    return (c * cfg.NPC_PAD + (l % 128) * cfg.NCH + (l // 128)).astype(np.int32)


def _pack(parts, dtype):
    """Pack [rows, cols] arrays into one [128, sum cols] array; return arr+offsets."""
    cols = sum(int(p.shape[1]) for p in parts.values())
    arr = np.zeros((128, cols), dtype)
    offs = {}
    o = 0
    for k, p in parts.items():
        r, c = p.shape
        arr[:r, o:o + c] = p
        offs[k] = (r, o, c)
        o += c
    return arr, offs


def prep(cfg, inputs):
    """Host-side sharding/layout. Returns (in_maps, cfg, pack_offs)."""
    f32 = np.float32
    x = np.asarray(inputs["x"], f32)
    edge_attr = np.asarray(inputs["edge_attr"], f32)
    edge_attr3 = np.asarray(inputs["edge_attr3"], f32)
    ei = np.asarray(inputs["edge_index"]).astype(np.int64)
    ei3 = np.asarray(inputs["edge_index3"]).astype(np.int64)

    W_node = np.asarray(inputs["W_node"], f32); b_node = np.asarray(inputs["b_node"], f32)
    W_ea = np.asarray(inputs["W_ea"], f32); b_ea = np.asarray(inputs["b_ea"], f32)
    W_e1 = np.asarray(inputs["W_e1"], f32); b_e1 = np.asarray(inputs["b_e1"], f32)
    W_e2 = np.asarray(inputs["W_e2"], f32); b_e2 = np.asarray(inputs["b_e2"], f32)
    conv_bias = np.asarray(inputs["conv_bias"], f32)
    W_ih = np.asarray(inputs["W_ih"], f32); b_ih = np.asarray(inputs["b_ih"], f32)
    W_hh = np.asarray(inputs["W_hh"], f32); b_hh = np.asarray(inputs["b_hh"], f32)
    W_l1 = np.asarray(inputs["W_l1"], f32); b_l1 = np.asarray(inputs["b_l1"], f32)
    W_l2 = np.asarray(inputs["W_l2"], f32); b_l2 = np.asarray(inputs["b_l2"], f32)

    D = cfg.DIM
    NC = cfg.NCORES
    src, dst = ei[0], ei[1]
    owner = dst // cfg.NPC

    # per-core edge shards sorted by (owner, dst)
    order_all = np.argsort(owner * cfg.N + dst, kind="stable")
    counts = np.bincount(owner, minlength=NC)
    offsets = np.concatenate([[0], np.cumsum(counts)])
    cfg.NT_E = max(1, math.ceil(int(counts.max()) / 128))
    cfg.EC_PAD = cfg.NT_E * 128
    cfg.has_be2 = bool(np.abs(b_e2).max() > 0)

    # f-major permutation of W_e2: W_e2p[k, f*64+d] = W_e2[k, d*64+f]
    W_e2p = W_e2.reshape(128, D, D).transpose(0, 2, 1).reshape(128, D * D).astype(BF)
    b_e2p = b_e2.reshape(D, D).T.reshape(1, D * D).astype(BF)

    packF_parts = {
        "bnode": b_node[:, None],
        "bea": b_ea[:, None],
        "be1": b_e1[:, None],
        "cbias": conv_bias[:, None],
        "Wih_rzT": W_ih[0:2 * D].T,
        "Wih_nT": W_ih[2 * D:3 * D].T,
        "Whh_rzT": W_hh[0:2 * D].T,
        "Whh_nT": W_hh[2 * D:3 * D].T,
        "br": (b_ih[0:D] + b_hh[0:D])[:, None],
        "bz": (b_ih[D:2 * D] + b_hh[D:2 * D])[:, None],
        "bin_": b_ih[2 * D:3 * D][:, None],
        "bhn": b_hh[2 * D:3 * D][:, None],
        "bl1": b_l1[:, None],
        "Wl2": W_l2,
        "bl2": b_l2[:, None],
    }
    packF, offF = _pack(packF_parts, f32)
    packB_parts = {
        "W_node": W_node.astype(BF),
        "W_ea": W_ea.astype(BF),
        "W_e1": W_e1.astype(BF),
        "Wl1a": (0.5 * W_l1[0:D]).astype(BF),
        "Wl1b": W_l1[D:].astype(BF),
        "ones1": np.ones((1, 128), BF),
    }
    if cfg.has_be2:
        packB_parts["be2p"] = b_e2p
    packB, offB = _pack(packB_parts, BF)
    pack_offs = (offF, offB)

    # full x in replica-row order (identical on every core): row
    # r = c*NPC_PAD + (l%128)*NCH + l//128  ->  node c*NPC + l
    NREP = NC * cfg.NPC_PAD
    rr = np.arange(NREP)
    c_ = rr // cfg.NPC_PAD
    i_ = rr % cfg.NPC_PAD
    l_ = (i_ % cfg.NCH) * 128 + i_ // cfg.NCH
    valid = l_ < cfg.NPC
    xTfull = np.zeros((x.shape[1], NREP), BF)
    xTfull[:, valid] = x[(c_ * cfg.NPC + l_)[valid]].T

    # local-src tiles: tiles whose edges all have src owned by this core can
    # gather from h_loc (ready before the AllGather).  NT_L is the number of
    # such tiles, uniform across cores (program structure is shared).
    n_local = [int(((src[order_all[offsets[c]:offsets[c + 1]]] // cfg.NPC) == c)
                   .sum()) for c in range(NC)]
    cfg.NT_L = min(n_local) // 128

    # pass 1: per-core edge order (local-src tiles first) + per-tile chunk sets
    per_core = []
    chunk_sets = [set() for _ in range(cfg.NT_E)]
    for c in range(NC):
        sel = order_all[offsets[c]:offsets[c + 1]]
        is_loc = (src[sel] // cfg.NPC) == c
        take = np.zeros(len(sel), bool)
        take[np.nonzero(is_loc)[0][:cfg.NT_L * 128]] = True
        edge_order = np.concatenate([sel[take], sel[~take]])
        ec = len(edge_order)
        dl = dst[edge_order] - c * cfg.NPC
        dl_pad = np.full(cfg.EC_PAD, 2**30, np.int64)
        dl_pad[:ec] = dl
        for t in range(cfg.NT_E):
            seg = dl_pad[t * 128:(t + 1) * 128]
            chunk_sets[t].update((seg[seg < cfg.NPC] // 128).tolist())
        per_core.append((edge_order, ec, dl, dl_pad))

    slots = []
    for t in range(cfg.NT_E):
        for c2 in sorted(chunk_sets[t]):
            slots.append((t, int(c2)))
    cfg.slots = tuple(slots)
    NS = len(slots)

    in_maps = []
    for c in range(NC):
        edge_order, ec, dl, dl_pad = per_core[c]
        e_src = src[edge_order]
        cnt = np.bincount(dl, minlength=cfg.NPC).astype(f32)
        cnt = np.maximum(cnt, 1.0)

        gsrc = np.zeros(cfg.EC_PAD, np.int32)
        gsrc[:ec] = _remap_node(cfg, e_src)
        assert NC * cfg.NPC_PAD < 2**15, "dma_gather needs int16 indices"
        # local row ids for the local-src tiles (gather from h_loc, k>0)
        gsrc_loc = np.zeros(max(cfg.NT_L, 1) * 128, np.int32)
        if cfg.NT_L > 0:
            sl = e_src[:cfg.NT_L * 128] - c * cfg.NPC
            gsrc_loc[:cfg.NT_L * 128] = ((sl % 128) * cfg.NCH +
                                         sl // 128).astype(np.int32)
        cntinv = np.zeros(cfg.EC_PAD, f32)
        cntinv[:ec] = 1.0 / cnt[dl]

        # shifted dst columns for the on-device selection build: [128, NS]
        tix = np.array([t for (t, c2) in slots], np.int64)
        c2s = np.array([c2 for (t, c2) in slots], np.int64)
        seg = dl_pad.reshape(cfg.NT_E, 128)
        dls = (seg[tix] - c2s[:, None] * 128).astype(f32).T.copy()  # [128, NS]

        eaT = np.zeros((edge_attr.shape[1], cfg.EC_PAD), BF)
        eaT[:, :ec] = edge_attr[edge_order].T

        xT = np.zeros((x.shape[1], cfg.NPC_PAD), BF)
        xT[:, :cfg.NPC] = x[c * cfg.NPC:(c + 1) * cfg.NPC].T

        sl3 = slice(c * cfg.E3PC, (c + 1) * cfg.E3PC)
        g3 = np.zeros((cfg.E3_PAD, 2), np.int32)
        g3[:cfg.E3PC, 0] = _remap_node(cfg, ei3[0, sl3])
        g3[:cfg.E3PC, 1] = _remap_node(cfg, ei3[1, sl3])
        g3 = g3.reshape(cfg.NT3, 128, 2).transpose(1, 0, 2).reshape(128, cfg.NT3 * 2)
        ea3T = np.zeros((edge_attr3.shape[1], cfg.E3_PAD), BF)
        ea3T[:, :cfg.E3PC] = edge_attr3[sl3].T

        m = {
            "packF": packF,
            "packB": packB,
            "W_e2ps": W_e2p[c * 16:(c + 1) * 16].copy(),
            "xT": xT,
            "xTfull": xTfull,
            "eaT": eaT,
            "ea3T": ea3T,
            "gsrc": gsrc.reshape(cfg.NT_E, 128).T.copy(),
            "dls": dls,
            "ci": cntinv.reshape(cfg.NT_E, 128).T.copy(),
            "g3": g3,
        }
        if cfg.NT_L > 0:
            m["gsrc_loc"] = gsrc_loc.reshape(max(cfg.NT_L, 1), 128).T.copy()
        in_maps.append(m)
    return in_maps, cfg, pack_offs


def _blocks(total, width):
    out = []
    o = 0
    while o < total:
        w = min(width, total - o)
        out.append((o, w))
        o += w
    return out


def build_program(cfg, pack_offs, sim1=False):
    D = cfg.DIM
    NC = cfg.NCORES
    offF, offB = pack_offs
    nc = bacc.Bacc("TRN2", target_bir_lowering=False, debug=False,
                   num_devices=1 if sim1 else NC)

    def din(name, shape, dt=F32):
        return nc.dram_tensor(name, shape, dt, kind="ExternalInput").ap()

    # ---- I/O declarations ----
    packF_in = din("packF", [128, sum(v[2] for v in offF.values())], F32)
    packB_in = din("packB", [128, sum(v[2] for v in offB.values())], BF16)
    W_e2ps_in = din("W_e2ps", [16, D * D], BF16)
    xT_in = din("xT", [8, cfg.NPC_PAD], BF16)
    xTfull_in = din("xTfull", [8, NC * cfg.NPC_PAD], BF16)
    eaT_in = din("eaT", [19, cfg.EC_PAD], BF16)
    ea3T_in = din("ea3T", [8, cfg.E3_PAD], BF16)
    NS = len(cfg.slots)
    gsrc_in = din("gsrc", [128, cfg.NT_E], I32)
    gsrc_loc_in = din("gsrc_loc", [128, cfg.NT_L], I32) if cfg.NT_L > 0 else None
    dls_in = din("dls", [128, NS], F32)
    ci_in = din("ci", [128, cfg.NT_E], F32)
    g3_in = din("g3", [128, cfg.NT3 * 2], I32)

    y_out = nc.dram_tensor("y", [1, cfg.E3_PAD], F32, kind="ExternalOutput").ap()

    NREP = NC * cfg.NPC_PAD

    with tile.TileContext(nc) as tc:
        # ---- DRAM internals ----
        w2_loc = tc.tile([16, D * D], BF16, space="DRAM", name="w2_loc")[0]
        w2_rep = tc.tile([16 * NC, D * D], BF16, space="DRAM",
                         addr_space="Shared", name="w2_rep")[0]
        h_loc = [None]
        h_rep = []
        for k in range(4):
            if k > 0:
                h_loc.append(tc.tile([cfg.NPC_PAD, D], BF16, space="DRAM",
                                     name=f"h_loc{k}")[0])
            h_rep.append(tc.tile([NREP, D], BF16, space="DRAM",
                                 addr_space="Shared", name=f"h_rep{k}")[0])

        # ---- persistent SBUF ----
        pers_cm = tc.tile_pool(name="pers", bufs=1)
        pers_p = pers_cm.__enter__()

        def load(name, ap_in):
            t = pers_p.tile(list(ap_in.shape), ap_in.dtype, name=name, tag=name)
            nc.sync.dma_start(out=t[:], in_=ap_in[:])
            return t

        packF_sb = load("packF_sb", packF_in)
        packB_sb = load("packB_sb", packB_in)
        gsrc_sb = load("gsrc_sb", gsrc_in)
        gsrc_loc_sb = (load("gsrc_loc_sb", gsrc_loc_in)
                       if gsrc_loc_in is not None else None)
        dls_sb = load("dls_sb", dls_in)
        ci_sb = load("ci_sb", ci_in)
        g3_sb = load("g3_sb", g3_in)

        def pF(key):
            r, o, cl = offF[key]
            return packF_sb[0:r, o:o + cl]

        def pB(key):
            r, o, cl = offB[key]
            return packB_sb[0:r, o:o + cl]

        W_e2p_sb = pers_p.tile([128, D * D], BF16, name="W_e2p_sb", tag="W_e2p_sb")
        S_sb = pers_p.tile([128, NS * 128], BF16,
                           name="S_sb", tag="S_sb")
        id_sb = pers_p.tile([128, 128], F32, name="id_sb", tag="id_sb")
        rh1T_sb = pers_p.tile([128, cfg.EC_PAD], BF16, name="rh1T_sb", tag="rh1T_sb")
        hTs = [pers_p.tile([D, cfg.NPC_PAD], F32, name=f"hT{k}", tag=f"hT{k}")
               for k in range(2)]
        hT = [hTs[0], hTs[1], hTs[0], hTs[1]]
        mT_sb = pers_p.tile([D, cfg.NPC_PAD], F32, name="mT_sb", tag="mT_sb")
        msg_sb = pers_p.tile([128, cfg.NT_E * D], BF16, name="msg_sb", tag="msg_sb")
        pairT_sb = pers_p.tile([D, cfg.E3_PAD], BF16, name="pairT_sb", tag="pairT_sb")
        hrow_sb = pers_p.tile([128, cfg.NCH * D], BF16, name="hrow_sb", tag="hrow_sb")
        iota_sb = pers_p.tile([128, 128], F32, name="iota_sb", tag="iota_sb")
        iotai_sb = pers_p.tile([128, 128], I32, name="iotai_sb", tag="iotai_sb")
        pcol_sb = pers_p.tile([128, 1], I32, name="pcol_sb", tag="pcol_sb")
        pcolf_sb = pers_p.tile([128, 1], F32, name="pcolf_sb", tag="pcolf_sb")

        # ---- startup: W_e2p AllGather, iota, identity, S build ----
        w2s_sb = pers_p.tile([16, D * D], BF16, name="w2s_sb", tag="w2s_sb")
        nc.sync.dma_start(out=w2s_sb[:], in_=W_e2ps_in[:])
        nc.sync.dma_start(out=w2_loc[:], in_=w2s_sb[:])
        if sim1 or cfg.no_coll:
            nc.sync.dma_start(out=w2_rep[0:16, :], in_=w2_loc[:])
        else:
            nc.gpsimd.collective_compute(
                "AllGather", ALU.bypass,
                replica_groups=[list(range(NC))],
                ins=[w2_loc[:].opt()],
                outs=[w2_rep[:].opt()],
            )
        nc.sync.dma_start(out=W_e2p_sb[:], in_=w2_rep[:])

        nc.gpsimd.iota(out=iotai_sb[:], pattern=[[1, 128]], base=0,
                       channel_multiplier=0)
        nc.vector.tensor_copy(out=iota_sb[:], in_=iotai_sb[:])
        nc.gpsimd.iota(out=pcol_sb[:], pattern=[[1, 1]], base=0,
                       channel_multiplier=1)
        nc.vector.tensor_copy(out=pcolf_sb[:], in_=pcol_sb[:])
        nc.vector.tensor_scalar(out=id_sb[:], in0=iota_sb[:],
                                scalar1=pcolf_sb[:], scalar2=None,
                                op0=ALU.is_equal)
        for sidx, (t, c2) in enumerate(cfg.slots):
            nc.vector.tensor_scalar(
                out=S_sb[:, sidx * 128:(sidx + 1) * 128], in0=iota_sb[:],
                scalar1=dls_sb[:, sidx:sidx + 1],
                scalar2=ci_sb[:, t:t + 1],
                op0=ALU.is_equal, op1=ALU.mult)

        # ---- pools ----
        with (
            tc.tile_pool(name="wpsum", bufs=2, space="PSUM") as wpsum_p,
            tc.tile_pool(name="sp", bufs=4, space="PSUM") as sp_p,
            tc.tile_pool(name="wedgep", bufs=2) as wedge_p,
            tc.tile_pool(name="tmpp", bufs=2) as tmp_p,
            tc.tile_pool(name="gath", bufs=4) as gath_p,
            tc.tile_pool(name="gruw", bufs=2) as gru_p,
            tc.tile_pool(name="strw", bufs=2) as str_p,
        ):
            # ablation constants
            zero64_sb = pers_p.tile([D, 128], F32, name="zero64", tag="zero64")
            nc.vector.memset(zero64_sb[:], 0)
            osrc_const = pers_p.tile([128, D], BF16, name="osrc_c", tag="osrc_c")
            wedge_const = pers_p.tile([128, D * D], BF16, name="wedge_c", tag="wedge_c")
            if not cfg.do_gather:
                nc.vector.memset(osrc_const[:], 0)
            if not cfg.do_wedge:
                nc.vector.memset(wedge_const[:], 0)
            if not cfg.do_apply:
                nc.vector.memset(msg_sb[:], 0)

            for _rep in range(cfg.REPS):
                # ---- edge MLP (once): rh1T = relu(W_e1^T @ relu(W_ea^T @ ea^T)) ----
                for (o, w) in _blocks(cfg.EC_PAD, 512):
                    eat_in = str_p.tile([19, 512], BF16, tag="eat_in")
                    nc.sync.dma_start(out=eat_in[:, :w], in_=eaT_in[:, o:o + w])
                    ps = sp_p.tile([128, 512], F32, tag="sp")
                    nc.tensor.matmul(out=ps[:12, :w], lhsT=pB("W_ea"),
                                     rhs=eat_in[:, :w], start=True, stop=True)
                    eat = str_p.tile([12, 512], BF16, tag="eat")
                    nc.scalar.activation(out=eat[:, :w], in_=ps[:12, :w],
                                         func=AF.Relu, bias=pF("bea"))
                    ps2 = sp_p.tile([128, 512], F32, tag="sp")
                    nc.tensor.matmul(out=ps2[:, :w], lhsT=pB("W_e1"), rhs=eat[:, :w],
                                     start=True, stop=True)
                    nc.scalar.activation(out=rh1T_sb[:, o:o + w], in_=ps2[:, :w],
                                         func=AF.Relu, bias=pF("be1"))

                # ---- node MLP: h0^T = relu(W_node^T @ x^T) ----
                for (o, w) in _blocks(cfg.NPC_PAD, 512):
                    xt_in = str_p.tile([8, 512], BF16, tag="xt_in")
                    nc.sync.dma_start(out=xt_in[:, :w], in_=xT_in[:, o:o + w])
                    ps = sp_p.tile([128, 512], F32, tag="sp")
                    nc.tensor.matmul(out=ps[:D, :w], lhsT=pB("W_node"),
                                     rhs=xt_in[:, :w], start=True, stop=True)
                    nc.scalar.activation(out=hT[0][:, o:o + w], in_=ps[:D, :w],
                                         func=AF.Relu, bias=pF("bnode"))

            # ---- helper: transpose hT -> rows, DMA, AllGather ----
                def publish(k):
                    for c2 in range(cfg.NCH):
                        tp = sp_p.tile([128, 512], F32, tag="sp")
                        nc.tensor.transpose(out=tp[:, :D],
                                            in_=hT[k][:, c2 * 128:(c2 + 1) * 128],
                                            identity=id_sb[:D, :D])
                        nc.scalar.activation(out=hrow_sb[:, c2 * D:(c2 + 1) * D],
                                             in_=tp[:, :D], func=AF.Copy)
                    nc.sync.dma_start(
                        out=h_loc[k][:].rearrange("(p c) d -> p (c d)", p=128),
                        in_=hrow_sb[:],
                    )
                    if sim1 or cfg.no_coll:
                        nc.sync.dma_start(out=h_rep[k][0:cfg.NPC_PAD, :],
                                          in_=h_loc[k][:])
                    else:
                        nc.gpsimd.collective_compute(
                            "AllGather", ALU.bypass,
                            replica_groups=[list(range(NC))],
                            ins=[h_loc[k][:].opt()],
                            outs=[h_rep[k][:].opt()],
                        )

                # ---- replicate h0 locally (h0 = relu(W_node^T x^T) needs no
                # edges, so every core computes the full replica itself --
                # identical bf16 values to a publish, without the AllGather) ----
                for g in range(NC):
                    for (o, w) in _blocks(cfg.NPC_PAD, 512):
                        xtf = str_p.tile([8, 512], BF16, tag="xt_in")
                        nc.sync.dma_start(
                            out=xtf[:, :w],
                            in_=xTfull_in[:, g * cfg.NPC_PAD + o:
                                          g * cfg.NPC_PAD + o + w])
                        ps = sp_p.tile([128, 512], F32, tag="sp")
                        nc.tensor.matmul(out=ps[:D, :w], lhsT=pB("W_node"),
                                         rhs=xtf[:, :w], start=True, stop=True)
                        h0g = gru_p.tile([D, 512], F32, tag="rt")
                        nc.scalar.activation(out=h0g[:, :w], in_=ps[:D, :w],
                                             func=AF.Relu, bias=pF("bnode"))
                        for q in range(0, w, 128):
                            c2 = (o + q) // 128
                            tp = sp_p.tile([128, 512], F32, tag="sp")
                            nc.tensor.transpose(out=tp[:, :D],
                                                in_=h0g[:, q:q + 128],
                                                identity=id_sb[:D, :D])
                            nc.scalar.activation(
                                out=hrow_sb[:, c2 * D:(c2 + 1) * D],
                                in_=tp[:, :D], func=AF.Copy)
                    nc.sync.dma_start(
                        out=h_rep[0][g * cfg.NPC_PAD:(g + 1) * cfg.NPC_PAD, :]
                        .rearrange("(p c) d -> p (c d)", p=128),
                        in_=hrow_sb[:],
                    )

                # ---- conv steps ----
                for k in range(3):
                    # msg phase
                    for t in range(cfg.NT_E):
                        if cfg.do_gather:
                            osrc_t = gath_p.tile([128, D], BF16, tag="osrc")
                            if k > 0 and t < cfg.NT_L:
                                nc.gpsimd.indirect_dma_start(
                                    out=osrc_t[:], out_offset=None,
                                    in_=h_loc[k][:],
                                    in_offset=bass.IndirectOffsetOnAxis(
                                        ap=gsrc_loc_sb[:, t:t + 1], axis=0),
                                )
                            else:
                                nc.gpsimd.indirect_dma_start(
                                    out=osrc_t[:], out_offset=None,
                                    in_=h_rep[k][:],
                                    in_offset=bass.IndirectOffsetOnAxis(
                                        ap=gsrc_sb[:, t:t + 1], axis=0),
                                )
                            osrc = osrc_t[:]
                        else:
                            osrc = osrc_const[:]
                        if cfg.do_wedge:
                            wedge = wedge_p.tile([128, D * D], BF16, tag="wedge")
                            for jh in range(4):
                                wp = wpsum_p.tile([128, 1024], F32, tag="wp")
                                for q in range(2):
                                    j = jh * 2 + q
                                    nc.tensor.matmul(
                                        out=wp[:, q * 512:(q + 1) * 512],
                                        lhsT=rh1T_sb[:, t * 128:(t + 1) * 128],
                                        rhs=W_e2p_sb[:, j * 512:(j + 1) * 512],
                                        start=True, stop=not cfg.has_be2)
                                    if cfg.has_be2:
                                        nc.tensor.matmul(
                                            out=wp[:, q * 512:(q + 1) * 512],
                                            lhsT=pB("ones1"),
                                            rhs=pB("be2p")[:, j * 512:(j + 1) * 512],
                                            start=False, stop=True)
                                nc.scalar.activation(
                                    out=wedge[:, jh * 1024:(jh + 1) * 1024],
                                    in_=wp[:], func=AF.Copy)
                            wedge_ap = wedge[:]
                        else:
                            wedge_ap = wedge_const[:]
                        if not cfg.do_apply:
                            continue
                        tmp = tmp_p.tile([128, D * D], BF16, tag="tmp")
                        tmp3 = tmp[:].rearrange("p (f d) -> p f d", d=D)
                        nc.vector.tensor_tensor(
                            out=tmp3,
                            in0=osrc.unsqueeze(1).to_broadcast([128, D, D]),
                            in1=wedge_ap.rearrange("p (f d) -> p f d", d=D),
                            op=ALU.mult)
                        for dd in (D // 2, D // 4, D // 8, D // 16, D // 32):
                            nc.vector.tensor_add(
                                out=tmp3[:, :, 0:dd], in0=tmp3[:, :, 0:dd],
                                in1=tmp3[:, :, dd:2 * dd])
                        nc.vector.tensor_add(
                            out=msg_sb[:, t * D:(t + 1) * D].unsqueeze(2),
                            in0=tmp3[:, :, 0:1], in1=tmp3[:, :, 1:2])

                    # scatter phase: aggT per node chunk
                    contributors = [[] for _ in range(cfg.NCH)]
                    for sidx, (t, c2) in enumerate(cfg.slots):
                        contributors[c2].append((t, sidx))
                    for c2 in range(cfg.NCH):
                        conts = contributors[c2]
                        if not conts:
                            nc.scalar.activation(
                                out=mT_sb[:, c2 * 128:(c2 + 1) * 128],
                                in_=zero64_sb[:], func=AF.Relu, bias=pF("cbias"))
                            continue
                        ap_ = sp_p.tile([128, 512], F32, tag="sp")
                        for j, (t, sidx) in enumerate(conts):
                            nc.tensor.matmul(
                                out=ap_[:D, :128], lhsT=msg_sb[:, t * D:(t + 1) * D],
                                rhs=S_sb[:, sidx * 128:(sidx + 1) * 128],
                                start=(j == 0), stop=(j == len(conts) - 1))
                        nc.scalar.activation(
                            out=mT_sb[:, c2 * 128:(c2 + 1) * 128], in_=ap_[:D, :128],
                            func=AF.Relu, bias=pF("cbias"))

                    # GRU phase
                    for (o, w) in _blocks(cfg.NPC_PAD, 512):
                        rp = sp_p.tile([128, 512], F32, tag="sp")
                        nc.tensor.matmul(out=rp[:D, :w], lhsT=pF("Wih_rzT")[:, :D],
                                         rhs=mT_sb[:, o:o + w], start=True, stop=False)
                        nc.tensor.matmul(out=rp[:D, :w], lhsT=pF("Whh_rzT")[:, :D],
                                         rhs=hT[k][:, o:o + w], start=False, stop=True)
                        rt = gru_p.tile([D, 512], F32, tag="rt")
                        nc.scalar.activation(out=rt[:, :w], in_=rp[:D, :w],
                                             func=AF.Sigmoid, bias=pF("br"))
                        zp = sp_p.tile([128, 512], F32, tag="sp")
                        nc.tensor.matmul(out=zp[:D, :w], lhsT=pF("Wih_rzT")[:, D:],
                                         rhs=mT_sb[:, o:o + w], start=True, stop=False)
                        nc.tensor.matmul(out=zp[:D, :w], lhsT=pF("Whh_rzT")[:, D:],
                                         rhs=hT[k][:, o:o + w], start=False, stop=True)
                        zt = gru_p.tile([D, 512], F32, tag="zt")
                        nc.scalar.activation(out=zt[:, :w], in_=zp[:D, :w],
                                             func=AF.Sigmoid, bias=pF("bz"))
                        np_ = sp_p.tile([128, 512], F32, tag="sp")
                        nc.tensor.matmul(out=np_[:D, :w], lhsT=pF("Wih_nT"),
                                         rhs=mT_sb[:, o:o + w], start=True, stop=True)
                        hnp = sp_p.tile([128, 512], F32, tag="sp")
                        nc.tensor.matmul(out=hnp[:D, :w], lhsT=pF("Whh_nT"),
                                         rhs=hT[k][:, o:o + w], start=True, stop=True)
                        hnb = gru_p.tile([D, 512], F32, tag="hnb")
                        nc.scalar.activation(out=hnb[:, :w], in_=hnp[:D, :w],
                                             func=AF.Identity, bias=pF("bhn"))
                        nc.vector.tensor_mul(out=hnb[:, :w], in0=rt[:, :w],
                                             in1=hnb[:, :w])
                        nc.vector.tensor_add(out=hnb[:, :w], in0=np_[:D, :w],
                                             in1=hnb[:, :w])
                        ng = gru_p.tile([D, 512], F32, tag="ng")
                        nc.scalar.activation(out=ng[:, :w], in_=hnb[:, :w],
                                             func=AF.Tanh, bias=pF("bin_"))
                        hmn = gru_p.tile([D, 512], F32, tag="hmn")
                        nc.vector.tensor_sub(out=hmn[:, :w], in0=hT[k][:, o:o + w],
                                             in1=ng[:, :w])
                        nc.vector.tensor_mul(out=hmn[:, :w], in0=zt[:, :w],
                                             in1=hmn[:, :w])
                        nc.vector.tensor_add(out=hT[k + 1][:, o:o + w], in0=ng[:, :w],
                                             in1=hmn[:, :w])

                    publish(k + 1)

                # ---- readout ----
                for t3 in range(cfg.NT3):
                    pa = gath_p.tile([128, D], BF16, tag="pa")
                    nc.gpsimd.indirect_dma_start(
                        out=pa[:], out_offset=None, in_=h_rep[3][:],
                        in_offset=bass.IndirectOffsetOnAxis(
                            ap=g3_sb[:, 2 * t3:2 * t3 + 1], axis=0))
                    pb = gath_p.tile([128, D], BF16, tag="pb")
                    nc.gpsimd.indirect_dma_start(
                        out=pb[:], out_offset=None, in_=h_rep[3][:],
                        in_offset=bass.IndirectOffsetOnAxis(
                            ap=g3_sb[:, 2 * t3 + 1:2 * t3 + 2], axis=0))
                    pab = gath_p.tile([128, D], F32, tag="pab")
                    nc.vector.tensor_add(out=pab[:], in0=pa[:], in1=pb[:])
                    tp = sp_p.tile([128, 512], F32, tag="sp")
                    nc.tensor.transpose(out=tp[:D, :128], in_=pab[:], identity=id_sb[:])
                    nc.scalar.activation(out=pairT_sb[:, t3 * 128:(t3 + 1) * 128],
                                         in_=tp[:D, :128], func=AF.Copy)

                for (o, w) in _blocks(cfg.E3_PAD, 512):
                    ea3t = str_p.tile([8, 512], BF16, tag="ea3t")
                    nc.sync.dma_start(out=ea3t[:, :w], in_=ea3T_in[:, o:o + w])
                    y1p = sp_p.tile([128, 512], F32, tag="sp")
                    nc.tensor.matmul(out=y1p[:, :w], lhsT=pB("Wl1a"),
                                     rhs=pairT_sb[:, o:o + w], start=True, stop=False)
                    nc.tensor.matmul(out=y1p[:, :w], lhsT=pB("Wl1b"),
                                     rhs=ea3t[:, :w], start=False, stop=True)
                    y1 = str_p.tile([128, 512], F32, tag="y1")
                    nc.scalar.activation(out=y1[:, :w], in_=y1p[:, :w],
                                         func=AF.Relu, bias=pF("bl1"))
                    yp = sp_p.tile([128, 512], F32, tag="sp")
                    nc.tensor.matmul(out=yp[:1, :w], lhsT=pF("Wl2"), rhs=y1[:, :w],
                                     start=True, stop=True)
                    yb = str_p.tile([1, 512], F32, tag="yb")
                    nc.scalar.activation(out=yb[:, :w], in_=yp[:1, :w],
                                         func=AF.Identity, bias=pF("bl2"))
                    nc.sync.dma_start(out=y_out[:, o:o + w], in_=yb[:, :w])

        pers_cm.__exit__(None, None, None)

    nc.compile()
    return nc


_PROG_CACHE = {}
_RUN_CACHE = {}


def _fingerprint(inputs):
    h = hashlib.blake2b(digest_size=16)
    for k in sorted(inputs):
        a = np.ascontiguousarray(np.asarray(inputs[k]))
        h.update(k.encode())
        h.update(str(a.shape).encode())
        h.update(str(a.dtype).encode())
        h.update(a.tobytes())
    return h.hexdigest()


def _get_program(cfg, inputs):
    in_maps, cfg, pack_offs = prep(cfg, inputs)
    key = (cfg.N, cfg.E, cfg.E3, cfg.NT_E, cfg.has_be2, cfg.REPS, cfg.NT_L,
           cfg.slots, cfg.do_wedge, cfg.do_apply, cfg.do_gather, cfg.no_coll)
    if key not in _PROG_CACHE:
        _PROG_CACHE[key] = build_program(cfg, pack_offs)
    return _PROG_CACHE[key], in_maps, cfg


def _pjrt_callable(nc, in_maps):
    """Build a cached jitted shard_map callable mirroring bass2jax's tail."""
    import jax
    from jax.sharding import Mesh, PartitionSpec
    from jax.experimental.shard_map import shard_map
    from concourse import bass2jax
    import concourse.mybir as mb

    bass2jax.install_neuronx_cc_hook()
    n_cores = len(in_maps)
    partition_name = nc.partition_id_tensor.name if nc.partition_id_tensor else None
    in_names, out_names, out_avals, zero_outs = [], [], [], []
    for alloc in nc.m.functions[0].allocations:
        if not isinstance(alloc, mb.MemoryLocationSet):
            continue
        name = alloc.memorylocations[0].name
        if alloc.kind == "ExternalInput":
            if name != partition_name:
                in_names.append(name)
        elif alloc.kind == "ExternalOutput":
            out_names.append(name)
            shape = tuple(alloc.tensor_shape)
            dtype = mb.dt.np(alloc.dtype)
            out_avals.append(jax.core.ShapedArray(shape, dtype))
            zero_outs.append(np.zeros(shape, dtype))
    n_params = len(in_names)
    n_outs = len(out_avals)
    in_names_full = list(in_names) + out_names
    if partition_name is not None:
        in_names_full.append(partition_name)
    donate = tuple(range(n_params, n_params + n_outs))

    def _body(*args):
        operands = list(args)
        if partition_name is not None:
            operands.append(bass2jax.partition_id_tensor())
        outs = bass2jax._bass_exec_p.bind(
            *operands,
            out_avals=tuple(out_avals),
            in_names=tuple(in_names_full),
            out_names=tuple(out_names),
            lowering_input_output_aliases=(),
            sim_require_finite=True,
            sim_require_nnan=True,
            nc=nc,
        )
        return tuple(outs)

    devices = jax.devices()[:n_cores]
    mesh = Mesh(np.array(devices), ("core",))
    in_specs = (PartitionSpec("core"),) * (n_params + n_outs)
    out_specs = (PartitionSpec("core"),) * len(out_names)
    sharded = jax.jit(
        shard_map(_body, mesh=mesh, in_specs=in_specs, out_specs=out_specs,
                  check_rep=False),
        donate_argnums=donate, keep_unused=True)
    concat_in = [np.concatenate([np.asarray(in_maps[c][nm]) for c in range(n_cores)],
                                axis=0) for nm in in_names]
    concat_zeros = [np.zeros((n_cores * z.shape[0], *z.shape[1:]), z.dtype)
                    for z in zero_outs]
    return sharded, concat_in, concat_zeros, out_names, out_avals


def _get_exec(inputs, cfg=None):
    """Cached (sharded, dev_in, concat_zeros, out meta, cfg) for these inputs."""
    import jax

    fp = _fingerprint(inputs)
    cfg = cfg or Cfg()
    rkey = (fp, cfg.REPS, cfg.do_wedge, cfg.do_apply, cfg.do_gather, cfg.no_coll)
    st = _RUN_CACHE.get(rkey)
    if st is None:
        nc, in_maps, cfg = _get_program(cfg, inputs)
        sharded, concat_in, concat_zeros, out_names, out_avals = _pjrt_callable(
            nc, in_maps)
        dev_in = [jax.device_put(a) for a in concat_in]
        for a in dev_in:
            a.block_until_ready()
        st = (sharded, dev_in, concat_zeros, out_names, out_avals, cfg)
        if len(_RUN_CACHE) > 4:
            _RUN_CACHE.clear()
        _RUN_CACHE[rkey] = st
    return st


def run_once(inputs, cfg=None):
    import jax

    sharded, dev_in, concat_zeros, out_names, out_avals, cfg = _get_exec(inputs, cfg)
    zeros = [jax.device_put(z) for z in concat_zeros]
    outs = sharded(*dev_in, *zeros)
    n_cores = cfg.NCORES
    yi = out_names.index("y")
    y = np.asarray(outs[yi]).reshape(n_cores, *out_avals[yi].shape)
    out = np.concatenate([y[c][0, :cfg.E3PC] for c in range(n_cores)])
    return out.astype(np.float32)


def kernel(**inputs) -> np.ndarray:
    return run_once(inputs)


# revision 27
# speedup vs baseline: 1.5074x; 1.5074x over previous
"""Trainium2 Bass kernel for nn_Net_12421045420310 (GNN edge-conditioned message passing).

Sharding (8 cores):
 - Nodes block-sharded: core c owns nodes [c*3125, (c+1)*3125).
 - Edges assigned to the core owning their dst node, sorted by dst within the
   shard -> scatter-mean is purely core-local; node state is re-replicated
   with one AllGather per conv step (bf16).
 - edge_index3/edge_attr3 position-sharded 5000/core; outputs stitched on host.

Device pipeline per conv step (per core):
 - Wedge ([128,64,64] per 128-edge tile) recomputed on TensorE from the
   SBUF-resident rh1^T (bf16) and a host-permuted f-major W_e2 (bf16, sharded
   across cores and AllGather'd on device at startup); never written to HBM.
 - out[src] rows gathered from a bf16 node-state replica via indirect DMA.
 - per-edge GEMV msg[e,f] = sum_d out_src[e,d]*Wedge[e,d,f] on VectorE:
   broadcast-AP tensor_tensor multiply + halving-add tree straight into bf16.
 - scatter-mean via selection-matrix matmuls on TensorE; the selection
   matrices are built on device (iota + per-partition is_equal compare) with
   1/cnt folded into their nonzeros, producing agg^T in [feat, node] layout.
 - GRU gates on PE/ScalarE/VectorE in transposed layout; h^T transposed back
   per 128-node chunk on TensorE, DMA'd to DRAM, AllGather.
"""

import hashlib
import math

import numpy as np
import ml_dtypes

import sys
import types

# This axon client build lacks antenv.axon_hooks; stub it so importing
# bass_utils under axon never trips on the optional profile hook.
if "antenv.axon_hooks" not in sys.modules:
    try:
        import antenv.axon_hooks  # noqa: F401
    except ImportError:
        _stub = types.ModuleType("antenv.axon_hooks")
        _stub.get_axon_ntff_profile_hook = lambda: None
        sys.modules["antenv.axon_hooks"] = _stub

import concourse.bass as bass
import concourse.bacc as bacc
import concourse.tile as tile
import concourse.mybir as mybir

AF = mybir.ActivationFunctionType
ALU = mybir.AluOpType
DT = mybir.dt

BF16 = DT.bfloat16
F32 = DT.float32
I32 = DT.int32

BF = ml_dtypes.bfloat16


class Cfg:
    def __init__(self, N=25000, E=50000, E3=40000, DIM=64, NCORES=8, K_SLOTS=3,
                 REPS=1, do_wedge=True, do_apply=True, do_gather=True,
                 no_coll=False, gp_every=0, dve_copy_every=0):
        assert N % NCORES == 0 and E3 % NCORES == 0 and DIM == 64
        self.N, self.E, self.E3, self.DIM, self.NCORES = N, E, E3, DIM, NCORES
        self.NPC = N // NCORES                      # nodes per core
        self.NCH = math.ceil(self.NPC / 128)        # node chunks per core
        self.NPC_PAD = self.NCH * 128
        self.E3PC = E3 // NCORES
        self.NT3 = math.ceil(self.E3PC / 128)
        self.E3_PAD = self.NT3 * 128
        self.K_SLOTS = K_SLOTS
        self.REPS = REPS
        self.do_wedge = do_wedge
        self.do_apply = do_apply
        self.do_gather = do_gather
        self.no_coll = no_coll
        self.gp_every = gp_every
        self.dve_copy_every = dve_copy_every
        # filled by prep():
        self.NT_E = None
        self.EC_PAD = None
        self.has_be2 = False

    def slot_chunk(self, t, slot):
        """Node chunk targeted by scatter slot (t, slot); identical across cores."""
        c = t * self.NCH // self.NT_E + slot - (self.K_SLOTS // 2)
        return min(max(c, 0), self.NCH - 1)


def _remap_node(cfg, n):
    """Map global node ids -> rows in the interleaved padded replica layout."""
    n = np.asarray(n, np.int64)
    c, l = np.divmod(n, cfg.NPC)
    return (c * cfg.NPC_PAD + (l % 128) * cfg.NCH + (l // 128)).astype(np.int32)


def _pack(parts, dtype):
    """Pack [rows, cols] arrays into one [128, sum cols] array; return arr+offsets."""
    cols = sum(int(p.shape[1]) for p in parts.values())
    arr = np.zeros((128, cols), dtype)
    offs = {}
    o = 0
    for k, p in parts.items():
        r, c = p.shape
        arr[:r, o:o + c] = p
        offs[k] = (r, o, c)
        o += c
    return arr, offs


def prep(cfg, inputs):
    """Host-side sharding/layout. Returns (in_maps, cfg, pack_offs)."""
    f32 = np.float32
    x = np.asarray(inputs["x"], f32)
    edge_attr = np.asarray(inputs["edge_attr"], f32)
    edge_attr3 = np.asarray(inputs["edge_attr3"], f32)
    ei = np.asarray(inputs["edge_index"]).astype(np.int64)
    ei3 = np.asarray(inputs["edge_index3"]).astype(np.int64)

    W_node = np.asarray(inputs["W_node"], f32); b_node = np.asarray(inputs["b_node"], f32)
    W_ea = np.asarray(inputs["W_ea"], f32); b_ea = np.asarray(inputs["b_ea"], f32)
    W_e1 = np.asarray(inputs["W_e1"], f32); b_e1 = np.asarray(inputs["b_e1"], f32)
    W_e2 = np.asarray(inputs["W_e2"], f32); b_e2 = np.asarray(inputs["b_e2"], f32)
    conv_bias = np.asarray(inputs["conv_bias"], f32)
    W_ih = np.asarray(inputs["W_ih"], f32); b_ih = np.asarray(inputs["b_ih"], f32)
    W_hh = np.asarray(inputs["W_hh"], f32); b_hh = np.asarray(inputs["b_hh"], f32)
    W_l1 = np.asarray(inputs["W_l1"], f32); b_l1 = np.asarray(inputs["b_l1"], f32)
    W_l2 = np.asarray(inputs["W_l2"], f32); b_l2 = np.asarray(inputs["b_l2"], f32)

    D = cfg.DIM
    NC = cfg.NCORES
    src, dst = ei[0], ei[1]
    owner = dst // cfg.NPC

    # per-core edge shards sorted by (owner, dst)
    order_all = np.argsort(owner * cfg.N + dst, kind="stable")
    counts = np.bincount(owner, minlength=NC)
    offsets = np.concatenate([[0], np.cumsum(counts)])
    cfg.NT_E = max(1, math.ceil(int(counts.max()) / 128))
    cfg.EC_PAD = cfg.NT_E * 128
    cfg.has_be2 = bool(np.abs(b_e2).max() > 0)

    # f-major permutation of W_e2: W_e2p[k, f*64+d] = W_e2[k, d*64+f]
    W_e2p = W_e2.reshape(128, D, D).transpose(0, 2, 1).reshape(128, D * D).astype(BF)
    b_e2p = b_e2.reshape(D, D).T.reshape(1, D * D).astype(BF)

    packF_parts = {
        "bnode": b_node[:, None],
        "bea": b_ea[:, None],
        "be1": b_e1[:, None],
        "cbias": conv_bias[:, None],
        "Wih_rzT": W_ih[0:2 * D].T,
        "Wih_nT": W_ih[2 * D:3 * D].T,
        "Whh_rzT": W_hh[0:2 * D].T,
        "Whh_nT": W_hh[2 * D:3 * D].T,
        "br": (b_ih[0:D] + b_hh[0:D])[:, None],
        "bz": (b_ih[D:2 * D] + b_hh[D:2 * D])[:, None],
        "bin_": b_ih[2 * D:3 * D][:, None],
        "bhn": b_hh[2 * D:3 * D][:, None],
        "bl1": b_l1[:, None],
        "Wl2": W_l2,
        "bl2": b_l2[:, None],
    }
    packF, offF = _pack(packF_parts, f32)
    packB_parts = {
        "W_node": W_node.astype(BF),
        "W_ea": W_ea.astype(BF),
        "W_e1": W_e1.astype(BF),
        "Wl1a": (0.5 * W_l1[0:D]).astype(BF),
        "Wl1b": W_l1[D:].astype(BF),
        "ones1": np.ones((1, 128), BF),
    }
    if cfg.has_be2:
        packB_parts["be2p"] = b_e2p
    packB, offB = _pack(packB_parts, BF)
    pack_offs = (offF, offB)

    # full x in replica-row order (identical on every core): row
    # r = c*NPC_PAD + (l%128)*NCH + l//128  ->  node c*NPC + l
    NREP = NC * cfg.NPC_PAD
    rr = np.arange(NREP)
    c_ = rr // cfg.NPC_PAD
    i_ = rr % cfg.NPC_PAD
    l_ = (i_ % cfg.NCH) * 128 + i_ // cfg.NCH
    valid = l_ < cfg.NPC
    xTfull = np.zeros((x.shape[1], NREP), BF)
    xTfull[:, valid] = x[(c_ * cfg.NPC + l_)[valid]].T

    # local-src tiles: tiles whose edges all have src owned by this core can
    # gather from h_loc (ready before the AllGather).  NT_L is the number of
    # such tiles, uniform across cores (program structure is shared).
    n_local = [int(((src[order_all[offsets[c]:offsets[c + 1]]] // cfg.NPC) == c)
                   .sum()) for c in range(NC)]
    cfg.NT_L = min(n_local) // 128

    # pass 1: per-core edge order (local-src tiles first) + per-tile chunk sets
    per_core = []
    chunk_sets = [set() for _ in range(cfg.NT_E)]
    for c in range(NC):
        sel = order_all[offsets[c]:offsets[c + 1]]
        is_loc = (src[sel] // cfg.NPC) == c
        take = np.zeros(len(sel), bool)
        take[np.nonzero(is_loc)[0][:cfg.NT_L * 128]] = True
        edge_order = np.concatenate([sel[take], sel[~take]])
        ec = len(edge_order)
        dl = dst[edge_order] - c * cfg.NPC
        dl_pad = np.full(cfg.EC_PAD, 2**30, np.int64)
        dl_pad[:ec] = dl
        for t in range(cfg.NT_E):
            seg = dl_pad[t * 128:(t + 1) * 128]
            chunk_sets[t].update((seg[seg < cfg.NPC] // 128).tolist())
        per_core.append((edge_order, ec, dl, dl_pad))

    slots = []
    for t in range(cfg.NT_E):
        for c2 in sorted(chunk_sets[t]):
            slots.append((t, int(c2)))
    cfg.slots = tuple(slots)
    NS = len(slots)

    in_maps = []
    for c in range(NC):
        edge_order, ec, dl, dl_pad = per_core[c]
        e_src = src[edge_order]
        cnt = np.bincount(dl, minlength=cfg.NPC).astype(f32)
        cnt = np.maximum(cnt, 1.0)

        gsrc = np.zeros(cfg.EC_PAD, np.int32)
        gsrc[:ec] = _remap_node(cfg, e_src)
        assert NC * cfg.NPC_PAD < 2**15, "dma_gather needs int16 indices"
        # local row ids for the local-src tiles (gather from h_loc, k>0)
        gsrc_loc = np.zeros(max(cfg.NT_L, 1) * 128, np.int32)
        if cfg.NT_L > 0:
            sl = e_src[:cfg.NT_L * 128] - c * cfg.NPC
            gsrc_loc[:cfg.NT_L * 128] = ((sl % 128) * cfg.NCH +
                                         sl // 128).astype(np.int32)
        cntinv = np.zeros(cfg.EC_PAD, f32)
        cntinv[:ec] = 1.0 / cnt[dl]

        # shifted dst columns for the on-device selection build: [128, NS]
        tix = np.array([t for (t, c2) in slots], np.int64)
        c2s = np.array([c2 for (t, c2) in slots], np.int64)
        seg = dl_pad.reshape(cfg.NT_E, 128)
        dls = (seg[tix] - c2s[:, None] * 128).astype(f32).T.copy()  # [128, NS]

        eaT = np.zeros((edge_attr.shape[1], cfg.EC_PAD), BF)
        eaT[:, :ec] = edge_attr[edge_order].T

        xT = np.zeros((x.shape[1], cfg.NPC_PAD), BF)
        xT[:, :cfg.NPC] = x[c * cfg.NPC:(c + 1) * cfg.NPC].T

        sl3 = slice(c * cfg.E3PC, (c + 1) * cfg.E3PC)
        g3 = np.zeros((cfg.E3_PAD, 2), np.int32)
        g3[:cfg.E3PC, 0] = _remap_node(cfg, ei3[0, sl3])
        g3[:cfg.E3PC, 1] = _remap_node(cfg, ei3[1, sl3])
        g3 = g3.reshape(cfg.NT3, 128, 2).transpose(1, 0, 2).reshape(128, cfg.NT3 * 2)
        ea3T = np.zeros((edge_attr3.shape[1], cfg.E3_PAD), BF)
        ea3T[:, :cfg.E3PC] = edge_attr3[sl3].T

        m = {
            "packF": packF,
            "packB": packB,
            "W_e2ps": W_e2p[c * 16:(c + 1) * 16].copy(),
            "xT": xT,
            "xTfull": xTfull,
            "eaT": eaT,
            "ea3T": ea3T,
            "gsrc": gsrc.reshape(cfg.NT_E, 128).T.copy(),
            "dls": dls,
            "ci": cntinv.reshape(cfg.NT_E, 128).T.copy(),
            "g3": g3,
        }
        if cfg.NT_L > 0:
            m["gsrc_loc"] = gsrc_loc.reshape(max(cfg.NT_L, 1), 128).T.copy()
        in_maps.append(m)
    return in_maps, cfg, pack_offs


def _blocks(total, width):
    out = []
    o = 0
    while o < total:
        w = min(width, total - o)
        out.append((o, w))
        o += w
    return out


def build_program(cfg, pack_offs, sim1=False):
    D = cfg.DIM
    NC = cfg.NCORES
    offF, offB = pack_offs
    nc = bacc.Bacc("TRN2", target_bir_lowering=False, debug=False,
                   num_devices=1 if sim1 else NC)

    def din(name, shape, dt=F32):
        return nc.dram_tensor(name, shape, dt, kind="ExternalInput").ap()

    # ---- I/O declarations ----
    packF_in = din("packF", [128, sum(v[2] for v in offF.values())], F32)
    packB_in = din("packB", [128, sum(v[2] for v in offB.values())], BF16)
    W_e2ps_in = din("W_e2ps", [16, D * D], BF16)
    xT_in = din("xT", [8, cfg.NPC_PAD], BF16)
    xTfull_in = din("xTfull", [8, NC * cfg.NPC_PAD], BF16)
    eaT_in = din("eaT", [19, cfg.EC_PAD], BF16)
    ea3T_in = din("ea3T", [8, cfg.E3_PAD], BF16)
    NS = len(cfg.slots)
    gsrc_in = din("gsrc", [128, cfg.NT_E], I32)
    gsrc_loc_in = din("gsrc_loc", [128, cfg.NT_L], I32) if cfg.NT_L > 0 else None
    dls_in = din("dls", [128, NS], F32)
    ci_in = din("ci", [128, cfg.NT_E], F32)
    g3_in = din("g3", [128, cfg.NT3 * 2], I32)

    y_out = nc.dram_tensor("y", [1, cfg.E3_PAD], F32, kind="ExternalOutput").ap()

    NREP = NC * cfg.NPC_PAD

    with tile.TileContext(nc) as tc:
        # ---- DRAM internals ----
        w2_loc = tc.tile([16, D * D], BF16, space="DRAM", name="w2_loc")[0]
        w2_rep = tc.tile([16 * NC, D * D], BF16, space="DRAM",
                         addr_space="Shared", name="w2_rep")[0]
        wedge_dram = tc.tile([cfg.NT_E * 128, D * D], BF16, space="DRAM",
                             name="wedge_dram")[0]
        h_loc = [None]
        h_rep = []
        for k in range(4):
            if k > 0:
                h_loc.append(tc.tile([cfg.NPC_PAD, D], BF16, space="DRAM",
                                     name=f"h_loc{k}")[0])
            h_rep.append(tc.tile([NREP, D], BF16, space="DRAM",
                                 addr_space="Shared", name=f"h_rep{k}")[0])

        # ---- persistent SBUF ----
        pers_cm = tc.tile_pool(name="pers", bufs=1)
        pers_p = pers_cm.__enter__()

        def load(name, ap_in):
            t = pers_p.tile(list(ap_in.shape), ap_in.dtype, name=name, tag=name)
            nc.sync.dma_start(out=t[:], in_=ap_in[:])
            return t

        packF_sb = load("packF_sb", packF_in)
        packB_sb = load("packB_sb", packB_in)
        gsrc_sb = load("gsrc_sb", gsrc_in)
        gsrc_loc_sb = (load("gsrc_loc_sb", gsrc_loc_in)
                       if gsrc_loc_in is not None else None)
        dls_sb = load("dls_sb", dls_in)
        ci_sb = load("ci_sb", ci_in)
        g3_sb = load("g3_sb", g3_in)

        def pF(key):
            r, o, cl = offF[key]
            return packF_sb[0:r, o:o + cl]

        def pB(key):
            r, o, cl = offB[key]
            return packB_sb[0:r, o:o + cl]

        W_e2p_sb = pers_p.tile([128, D * D], BF16, name="W_e2p_sb", tag="W_e2p_sb")
        S_sb = pers_p.tile([128, NS * 128], BF16,
                           name="S_sb", tag="S_sb")
        id_sb = pers_p.tile([128, 128], F32, name="id_sb", tag="id_sb")
        rh1T_sb = pers_p.tile([128, cfg.EC_PAD], BF16, name="rh1T_sb", tag="rh1T_sb")
        hTs = [pers_p.tile([D, cfg.NPC_PAD], F32, name=f"hT{k}", tag=f"hT{k}")
               for k in range(2)]
        hT = [hTs[0], hTs[1], hTs[0], hTs[1]]
        mT_sb = pers_p.tile([D, cfg.NPC_PAD], F32, name="mT_sb", tag="mT_sb")
        msg_sb = pers_p.tile([128, cfg.NT_E * D], BF16, name="msg_sb", tag="msg_sb")
        pairT_sb = pers_p.tile([D, cfg.E3_PAD], BF16, name="pairT_sb", tag="pairT_sb")
        hrow_sb = pers_p.tile([128, cfg.NCH * D], BF16, name="hrow_sb", tag="hrow_sb")
        iota_sb = pers_p.tile([128, 128], F32, name="iota_sb", tag="iota_sb")
        iotai_sb = pers_p.tile([128, 128], I32, name="iotai_sb", tag="iotai_sb")
        pcol_sb = pers_p.tile([128, 1], I32, name="pcol_sb", tag="pcol_sb")
        pcolf_sb = pers_p.tile([128, 1], F32, name="pcolf_sb", tag="pcolf_sb")

        # ---- startup: W_e2p AllGather, iota, identity, S build ----
        w2s_sb = pers_p.tile([16, D * D], BF16, name="w2s_sb", tag="w2s_sb")
        nc.sync.dma_start(out=w2s_sb[:], in_=W_e2ps_in[:])
        nc.sync.dma_start(out=w2_loc[:], in_=w2s_sb[:])
        if sim1 or cfg.no_coll:
            nc.sync.dma_start(out=w2_rep[0:16, :], in_=w2_loc[:])
        else:
            nc.gpsimd.collective_compute(
                "AllGather", ALU.bypass,
                replica_groups=[list(range(NC))],
                ins=[w2_loc[:].opt()],
                outs=[w2_rep[:].opt()],
            )
        nc.sync.dma_start(out=W_e2p_sb[:], in_=w2_rep[:])

        nc.gpsimd.iota(out=iotai_sb[:], pattern=[[1, 128]], base=0,
                       channel_multiplier=0)
        nc.vector.tensor_copy(out=iota_sb[:], in_=iotai_sb[:])
        nc.gpsimd.iota(out=pcol_sb[:], pattern=[[1, 1]], base=0,
                       channel_multiplier=1)
        nc.vector.tensor_copy(out=pcolf_sb[:], in_=pcol_sb[:])
        nc.vector.tensor_scalar(out=id_sb[:], in0=iota_sb[:],
                                scalar1=pcolf_sb[:], scalar2=None,
                                op0=ALU.is_equal)
        for sidx, (t, c2) in enumerate(cfg.slots):
            nc.vector.tensor_scalar(
                out=S_sb[:, sidx * 128:(sidx + 1) * 128], in0=iota_sb[:],
                scalar1=dls_sb[:, sidx:sidx + 1],
                scalar2=ci_sb[:, t:t + 1],
                op0=ALU.is_equal, op1=ALU.mult)

        # ---- pools ----
        with (
            tc.tile_pool(name="wpsum", bufs=2, space="PSUM") as wpsum_p,
            tc.tile_pool(name="sp", bufs=4, space="PSUM") as sp_p,
            tc.tile_pool(name="wedgep", bufs=3) as wedge_p,
            tc.tile_pool(name="tmpp", bufs=2) as tmp_p,
            tc.tile_pool(name="gath", bufs=4) as gath_p,
            tc.tile_pool(name="gruw", bufs=1) as gru_p,
            tc.tile_pool(name="strw", bufs=2) as str_p,
        ):
            # ablation constants
            zero64_sb = pers_p.tile([D, 128], F32, name="zero64", tag="zero64")
            nc.vector.memset(zero64_sb[:], 0)
            osrc_const = pers_p.tile([128, D], BF16, name="osrc_c", tag="osrc_c")
            wedge_const = pers_p.tile([128, D * D], BF16, name="wedge_c", tag="wedge_c")
            if not cfg.do_gather:
                nc.vector.memset(osrc_const[:], 0)
            if not cfg.do_wedge:
                nc.vector.memset(wedge_const[:], 0)
            if not cfg.do_apply:
                nc.vector.memset(msg_sb[:], 0)

            for _rep in range(cfg.REPS):
                # ---- edge MLP (once): rh1T = relu(W_e1^T @ relu(W_ea^T @ ea^T)) ----
                for (o, w) in _blocks(cfg.EC_PAD, 512):
                    eat_in = str_p.tile([19, 512], BF16, tag="eat_in")
                    nc.sync.dma_start(out=eat_in[:, :w], in_=eaT_in[:, o:o + w])
                    ps = sp_p.tile([128, 512], F32, tag="sp")
                    nc.tensor.matmul(out=ps[:12, :w], lhsT=pB("W_ea"),
                                     rhs=eat_in[:, :w], start=True, stop=True)
                    eat = str_p.tile([12, 512], BF16, tag="eat")
                    nc.scalar.activation(out=eat[:, :w], in_=ps[:12, :w],
                                         func=AF.Relu, bias=pF("bea"))
                    ps2 = sp_p.tile([128, 512], F32, tag="sp")
                    nc.tensor.matmul(out=ps2[:, :w], lhsT=pB("W_e1"), rhs=eat[:, :w],
                                     start=True, stop=True)
                    nc.scalar.activation(out=rh1T_sb[:, o:o + w], in_=ps2[:, :w],
                                         func=AF.Relu, bias=pF("be1"))

                # ---- node MLP: h0^T = relu(W_node^T @ x^T) ----
                for (o, w) in _blocks(cfg.NPC_PAD, 512):
                    xt_in = str_p.tile([8, 512], BF16, tag="xt_in")
                    nc.sync.dma_start(out=xt_in[:, :w], in_=xT_in[:, o:o + w])
                    ps = sp_p.tile([128, 512], F32, tag="sp")
                    nc.tensor.matmul(out=ps[:D, :w], lhsT=pB("W_node"),
                                     rhs=xt_in[:, :w], start=True, stop=True)
                    nc.scalar.activation(out=hT[0][:, o:o + w], in_=ps[:D, :w],
                                         func=AF.Relu, bias=pF("bnode"))

            # ---- helper: transpose hT -> rows, DMA, AllGather ----
                def publish(k):
                    for c2 in range(cfg.NCH):
                        tp = sp_p.tile([128, 512], F32, tag="sp")
                        nc.tensor.transpose(out=tp[:, :D],
                                            in_=hT[k][:, c2 * 128:(c2 + 1) * 128],
                                            identity=id_sb[:D, :D])
                        nc.scalar.activation(out=hrow_sb[:, c2 * D:(c2 + 1) * D],
                                             in_=tp[:, :D], func=AF.Copy)
                    nc.sync.dma_start(
                        out=h_loc[k][:].rearrange("(p c) d -> p (c d)", p=128),
                        in_=hrow_sb[:],
                    )
                    if sim1 or cfg.no_coll:
                        nc.sync.dma_start(out=h_rep[k][0:cfg.NPC_PAD, :],
                                          in_=h_loc[k][:])
                    else:
                        nc.gpsimd.collective_compute(
                            "AllGather", ALU.bypass,
                            replica_groups=[list(range(NC))],
                            ins=[h_loc[k][:].opt()],
                            outs=[h_rep[k][:].opt()],
                        )

                # ---- replicate h0 locally (h0 = relu(W_node^T x^T) needs no
                # edges, so every core computes the full replica itself --
                # identical bf16 values to a publish, without the AllGather) ----
                for g in range(NC):
                    for (o, w) in _blocks(cfg.NPC_PAD, 512):
                        xtf = str_p.tile([8, 512], BF16, tag="xt_in")
                        nc.sync.dma_start(
                            out=xtf[:, :w],
                            in_=xTfull_in[:, g * cfg.NPC_PAD + o:
                                          g * cfg.NPC_PAD + o + w])
                        ps = sp_p.tile([128, 512], F32, tag="sp")
                        nc.tensor.matmul(out=ps[:D, :w], lhsT=pB("W_node"),
                                         rhs=xtf[:, :w], start=True, stop=True)
                        h0g = gru_p.tile([D, 512], F32, tag="rt")
                        nc.scalar.activation(out=h0g[:, :w], in_=ps[:D, :w],
                                             func=AF.Relu, bias=pF("bnode"))
                        for q in range(0, w, 128):
                            c2 = (o + q) // 128
                            tp = sp_p.tile([128, 512], F32, tag="sp")
                            nc.tensor.transpose(out=tp[:, :D],
                                                in_=h0g[:, q:q + 128],
                                                identity=id_sb[:D, :D])
                            nc.scalar.activation(
                                out=hrow_sb[:, c2 * D:(c2 + 1) * D],
                                in_=tp[:, :D], func=AF.Copy)
                    nc.sync.dma_start(
                        out=h_rep[0][g * cfg.NPC_PAD:(g + 1) * cfg.NPC_PAD, :]
                        .rearrange("(p c) d -> p (c d)", p=128),
                        in_=hrow_sb[:],
                    )

                # ---- conv steps ----
                for k in range(3):
                    # msg phase
                    for t in range(cfg.NT_E):
                        if cfg.do_gather:
                            osrc_t = gath_p.tile([128, D], BF16, tag="osrc")
                            if k > 0 and t < cfg.NT_L:
                                nc.gpsimd.indirect_dma_start(
                                    out=osrc_t[:], out_offset=None,
                                    in_=h_loc[k][:],
                                    in_offset=bass.IndirectOffsetOnAxis(
                                        ap=gsrc_loc_sb[:, t:t + 1], axis=0),
                                )
                            else:
                                nc.gpsimd.indirect_dma_start(
                                    out=osrc_t[:], out_offset=None,
                                    in_=h_rep[k][:],
                                    in_offset=bass.IndirectOffsetOnAxis(
                                        ap=gsrc_sb[:, t:t + 1], axis=0),
                                )
                            osrc = osrc_t[:]
                        else:
                            osrc = osrc_const[:]
                        if cfg.do_wedge and k == 0:
                            # compute Wedge on PE once, spill to DRAM
                            wedge = wedge_p.tile([128, D * D], BF16, tag="wedge")
                            for jh in range(4):
                                wp = wpsum_p.tile([128, 1024], F32, tag="wp")
                                for q in range(2):
                                    j = jh * 2 + q
                                    nc.tensor.matmul(
                                        out=wp[:, q * 512:(q + 1) * 512],
                                        lhsT=rh1T_sb[:, t * 128:(t + 1) * 128],
                                        rhs=W_e2p_sb[:, j * 512:(j + 1) * 512],
                                        start=True, stop=not cfg.has_be2)
                                    if cfg.has_be2:
                                        nc.tensor.matmul(
                                            out=wp[:, q * 512:(q + 1) * 512],
                                            lhsT=pB("ones1"),
                                            rhs=pB("be2p")[:, j * 512:(j + 1) * 512],
                                            start=False, stop=True)
                                dce = cfg.dve_copy_every
                                if dce > 0 and (t * 4 + jh) % dce == dce - 1:
                                    nc.vector.tensor_copy(
                                        out=wedge[:, jh * 1024:(jh + 1) * 1024],
                                        in_=wp[:])
                                else:
                                    nc.scalar.activation(
                                        out=wedge[:, jh * 1024:(jh + 1) * 1024],
                                        in_=wp[:], func=AF.Copy)
                            nc.sync.dma_start(
                                out=wedge_dram[t * 128:(t + 1) * 128, :],
                                in_=wedge[:])
                            wedge_ap = wedge[:]
                        elif cfg.do_wedge:
                            # steps 1-2: stream the spilled Wedge back in
                            wedge = wedge_p.tile([128, D * D], BF16, tag="wedge")
                            nc.sync.dma_start(
                                out=wedge[:],
                                in_=wedge_dram[t * 128:(t + 1) * 128, :])
                            wedge_ap = wedge[:]
                        else:
                            wedge_ap = wedge_const[:]
                        if not cfg.do_apply:
                            continue
                        tmp = tmp_p.tile([128, D * D], BF16, tag="tmp")
                        tmp3 = tmp[:].rearrange("p (f d) -> p f d", d=D)
                        nc.vector.tensor_tensor(
                            out=tmp3,
                            in0=osrc.unsqueeze(1).to_broadcast([128, D, D]),
                            in1=wedge_ap.rearrange("p (f d) -> p f d", d=D),
                            op=ALU.mult)
                        # offload the add-tree of some tiles to GpSimd to
                        # relieve the (bottleneck) vector engine
                        on_gp = (cfg.gp_every > 0 and t >= cfg.NT_L
                                 and t % cfg.gp_every == cfg.gp_every - 1)
                        eng = nc.gpsimd if on_gp else nc.vector
                        for dd in (D // 2, D // 4, D // 8, D // 16, D // 32):
                            eng.tensor_tensor(
                                out=tmp3[:, :, 0:dd], in0=tmp3[:, :, 0:dd],
                                in1=tmp3[:, :, dd:2 * dd], op=ALU.add)
                        eng.tensor_tensor(
                            out=msg_sb[:, t * D:(t + 1) * D].unsqueeze(2),
                            in0=tmp3[:, :, 0:1], in1=tmp3[:, :, 1:2], op=ALU.add)

                    # scatter phase: aggT per node chunk
                    contributors = [[] for _ in range(cfg.NCH)]
                    for sidx, (t, c2) in enumerate(cfg.slots):
                        contributors[c2].append((t, sidx))
                    for c2 in range(cfg.NCH):
                        conts = contributors[c2]
                        if not conts:
                            nc.scalar.activation(
                                out=mT_sb[:, c2 * 128:(c2 + 1) * 128],
                                in_=zero64_sb[:], func=AF.Relu, bias=pF("cbias"))
                            continue
                        ap_ = sp_p.tile([128, 512], F32, tag="sp")
                        for j, (t, sidx) in enumerate(conts):
                            nc.tensor.matmul(
                                out=ap_[:D, :128], lhsT=msg_sb[:, t * D:(t + 1) * D],
                                rhs=S_sb[:, sidx * 128:(sidx + 1) * 128],
                                start=(j == 0), stop=(j == len(conts) - 1))
                        nc.scalar.activation(
                            out=mT_sb[:, c2 * 128:(c2 + 1) * 128], in_=ap_[:D, :128],
                            func=AF.Relu, bias=pF("cbias"))

                    # GRU phase
                    for (o, w) in _blocks(cfg.NPC_PAD, 512):
                        rp = sp_p.tile([128, 512], F32, tag="sp")
                        nc.tensor.matmul(out=rp[:D, :w], lhsT=pF("Wih_rzT")[:, :D],
                                         rhs=mT_sb[:, o:o + w], start=True, stop=False)
                        nc.tensor.matmul(out=rp[:D, :w], lhsT=pF("Whh_rzT")[:, :D],
                                         rhs=hT[k][:, o:o + w], start=False, stop=True)
                        rt = gru_p.tile([D, 512], F32, tag="rt")
                        nc.scalar.activation(out=rt[:, :w], in_=rp[:D, :w],
                                             func=AF.Sigmoid, bias=pF("br"))
                        zp = sp_p.tile([128, 512], F32, tag="sp")
                        nc.tensor.matmul(out=zp[:D, :w], lhsT=pF("Wih_rzT")[:, D:],
                                         rhs=mT_sb[:, o:o + w], start=True, stop=False)
                        nc.tensor.matmul(out=zp[:D, :w], lhsT=pF("Whh_rzT")[:, D:],
                                         rhs=hT[k][:, o:o + w], start=False, stop=True)
                        zt = gru_p.tile([D, 512], F32, tag="zt")
                        nc.scalar.activation(out=zt[:, :w], in_=zp[:D, :w],
                                             func=AF.Sigmoid, bias=pF("bz"))
                        np_ = sp_p.tile([128, 512], F32, tag="sp")
                        nc.tensor.matmul(out=np_[:D, :w], lhsT=pF("Wih_nT"),
                                         rhs=mT_sb[:, o:o + w], start=True, stop=True)
                        hnp = sp_p.tile([128, 512], F32, tag="sp")
                        nc.tensor.matmul(out=hnp[:D, :w], lhsT=pF("Whh_nT"),
                                         rhs=hT[k][:, o:o + w], start=True, stop=True)
                        hnb = gru_p.tile([D, 512], F32, tag="hnb")
                        nc.scalar.activation(out=hnb[:, :w], in_=hnp[:D, :w],
                                             func=AF.Identity, bias=pF("bhn"))
                        nc.vector.tensor_mul(out=hnb[:, :w], in0=rt[:, :w],
                                             in1=hnb[:, :w])
                        nc.vector.tensor_add(out=hnb[:, :w], in0=np_[:D, :w],
                                             in1=hnb[:, :w])
                        ng = gru_p.tile([D, 512], F32, tag="ng")
                        nc.scalar.activation(out=ng[:, :w], in_=hnb[:, :w],
                                             func=AF.Tanh, bias=pF("bin_"))
                        hmn = gru_p.tile([D, 512], F32, tag="hmn")
                        nc.vector.tensor_sub(out=hmn[:, :w], in0=hT[k][:, o:o + w],
                                             in1=ng[:, :w])
                        nc.vector.tensor_mul(out=hmn[:, :w], in0=zt[:, :w],
                                             in1=hmn[:, :w])
                        nc.vector.tensor_add(out=hT[k + 1][:, o:o + w], in0=ng[:, :w],
                                             in1=hmn[:, :w])

                    publish(k + 1)

                # ---- readout ----
                for t3 in range(cfg.NT3):
                    pa = gath_p.tile([128, D], BF16, tag="pa")
                    nc.gpsimd.indirect_dma_start(
                        out=pa[:], out_offset=None, in_=h_rep[3][:],
                        in_offset=bass.IndirectOffsetOnAxis(
                            ap=g3_sb[:, 2 * t3:2 * t3 + 1], axis=0))
                    pb = gath_p.tile([128, D], BF16, tag="pb")
                    nc.gpsimd.indirect_dma_start(
                        out=pb[:], out_offset=None, in_=h_rep[3][:],
                        in_offset=bass.IndirectOffsetOnAxis(
                            ap=g3_sb[:, 2 * t3 + 1:2 * t3 + 2], axis=0))
                    pab = gath_p.tile([128, D], F32, tag="pab")
                    nc.vector.tensor_add(out=pab[:], in0=pa[:], in1=pb[:])
                    tp = sp_p.tile([128, 512], F32, tag="sp")
                    nc.tensor.transpose(out=tp[:D, :128], in_=pab[:], identity=id_sb[:])
                    nc.scalar.activation(out=pairT_sb[:, t3 * 128:(t3 + 1) * 128],
                                         in_=tp[:D, :128], func=AF.Copy)

                for (o, w) in _blocks(cfg.E3_PAD, 512):
                    ea3t = str_p.tile([8, 512], BF16, tag="ea3t")
                    nc.sync.dma_start(out=ea3t[:, :w], in_=ea3T_in[:, o:o + w])
                    y1p = sp_p.tile([128, 512], F32, tag="sp")
                    nc.tensor.matmul(out=y1p[:, :w], lhsT=pB("Wl1a"),
                                     rhs=pairT_sb[:, o:o + w], start=True, stop=False)
                    nc.tensor.matmul(out=y1p[:, :w], lhsT=pB("Wl1b"),
                                     rhs=ea3t[:, :w], start=False, stop=True)
                    y1 = str_p.tile([128, 512], F32, tag="y1")
                    nc.scalar.activation(out=y1[:, :w], in_=y1p[:, :w],
                                         func=AF.Relu, bias=pF("bl1"))
                    yp = sp_p.tile([128, 512], F32, tag="sp")
                    nc.tensor.matmul(out=yp[:1, :w], lhsT=pF("Wl2"), rhs=y1[:, :w],
                                     start=True, stop=True)
                    yb = str_p.tile([1, 512], F32, tag="yb")
                    nc.scalar.activation(out=yb[:, :w], in_=yp[:1, :w],
                                         func=AF.Identity, bias=pF("bl2"))
                    nc.sync.dma_start(out=y_out[:, o:o + w], in_=yb[:, :w])

        pers_cm.__exit__(None, None, None)

    nc.compile()
    return nc


_PROG_CACHE = {}
_RUN_CACHE = {}


def _fingerprint(inputs):
    h = hashlib.blake2b(digest_size=16)
    for k in sorted(inputs):
        a = np.ascontiguousarray(np.asarray(inputs[k]))
        h.update(k.encode())
        h.update(str(a.shape).encode())
        h.update(str(a.dtype).encode())
        h.update(a.tobytes())
    return h.hexdigest()


def _get_program(cfg, inputs):
    in_maps, cfg, pack_offs = prep(cfg, inputs)
    key = (cfg.N, cfg.E, cfg.E3, cfg.NT_E, cfg.has_be2, cfg.REPS, cfg.NT_L,
           cfg.slots, cfg.do_wedge, cfg.do_apply, cfg.do_gather, cfg.no_coll,
           cfg.gp_every, cfg.dve_copy_every)
    if key not in _PROG_CACHE:
        _PROG_CACHE[key] = build_program(cfg, pack_offs)
    return _PROG_CACHE[key], in_maps, cfg


def _pjrt_callable(nc, in_maps):
    """Build a cached jitted shard_map callable mirroring bass2jax's tail."""
    import jax
    from jax.sharding import Mesh, PartitionSpec
    from jax.experimental.shard_map import shard_map
    from concourse import bass2jax
    import concourse.mybir as mb

    bass2jax.install_neuronx_cc_hook()
    n_cores = len(in_maps)
    partition_name = nc.partition_id_tensor.name if nc.partition_id_tensor else None
    in_names, out_names, out_avals, zero_outs = [], [], [], []
    for alloc in nc.m.functions[0].allocations:
        if not isinstance(alloc, mb.MemoryLocationSet):
            continue
        name = alloc.memorylocations[0].name
        if alloc.kind == "ExternalInput":
            if name != partition_name:
                in_names.append(name)
        elif alloc.kind == "ExternalOutput":
            out_names.append(name)
            shape = tuple(alloc.tensor_shape)
            dtype = mb.dt.np(alloc.dtype)
            out_avals.append(jax.core.ShapedArray(shape, dtype))
            zero_outs.append(np.zeros(shape, dtype))
    n_params = len(in_names)
    n_outs = len(out_avals)
    in_names_full = list(in_names) + out_names
    if partition_name is not None:
        in_names_full.append(partition_name)
    donate = tuple(range(n_params, n_params + n_outs))

    def _body(*args):
        operands = list(args)
        if partition_name is not None:
            operands.append(bass2jax.partition_id_tensor())
        outs = bass2jax._bass_exec_p.bind(
            *operands,
            out_avals=tuple(out_avals),
            in_names=tuple(in_names_full),
            out_names=tuple(out_names),
            lowering_input_output_aliases=(),
            sim_require_finite=True,
            sim_require_nnan=True,
            nc=nc,
        )
        return tuple(outs)

    devices = jax.devices()[:n_cores]
    mesh = Mesh(np.array(devices), ("core",))
    in_specs = (PartitionSpec("core"),) * (n_params + n_outs)
    out_specs = (PartitionSpec("core"),) * len(out_names)
    sharded = jax.jit(
        shard_map(_body, mesh=mesh, in_specs=in_specs, out_specs=out_specs,
                  check_rep=False),
        donate_argnums=donate, keep_unused=True)
    concat_in = [np.concatenate([np.asarray(in_maps[c][nm]) for c in range(n_cores)],
                                axis=0) for nm in in_names]
    concat_zeros = [np.zeros((n_cores * z.shape[0], *z.shape[1:]), z.dtype)
                    for z in zero_outs]
    return sharded, concat_in, concat_zeros, out_names, out_avals


def _get_exec(inputs, cfg=None):
    """Cached (sharded, dev_in, concat_zeros, out meta, cfg) for these inputs."""
    import jax

    fp = _fingerprint(inputs)
    cfg = cfg or Cfg()
    rkey = (fp, cfg.REPS, cfg.do_wedge, cfg.do_apply, cfg.do_gather, cfg.no_coll,
            cfg.gp_every, cfg.dve_copy_every)
    st = _RUN_CACHE.get(rkey)
    if st is None:
        nc, in_maps, cfg = _get_program(cfg, inputs)
        sharded, concat_in, concat_zeros, out_names, out_avals = _pjrt_callable(
            nc, in_maps)
        dev_in = [jax.device_put(a) for a in concat_in]
        for a in dev_in:
            a.block_until_ready()
        st = (sharded, dev_in, concat_zeros, out_names, out_avals, cfg)
        if len(_RUN_CACHE) > 4:
            _RUN_CACHE.clear()
        _RUN_CACHE[rkey] = st
    return st


def run_once(inputs, cfg=None):
    import jax

    sharded, dev_in, concat_zeros, out_names, out_avals, cfg = _get_exec(inputs, cfg)
    zeros = [jax.device_put(z) for z in concat_zeros]
    outs = sharded(*dev_in, *zeros)
    n_cores = cfg.NCORES
    yi = out_names.index("y")
    y = np.asarray(outs[yi]).reshape(n_cores, *out_avals[yi].shape)
    out = np.concatenate([y[c][0, :cfg.E3PC] for c in range(n_cores)])
    return out.astype(np.float32)


def kernel(**inputs) -> np.ndarray:
    return run_once(inputs)
